# revision 42
# baseline (speedup 1.0000x reference)
"""Distortion-regularization loss on Trainium2 (8 NeuronCores, SPMD).

Math: the reference loss collapses to a single quadratic form
    loss = mean_n( w_n^T A w_n ),   A = |u_i - u_j| + diag(ds)/3   (32x32 const)
         = <A, W^T W> / N_RAYS
so each core only needs the Gram matrix of its ray shard:
    Gram_c = W_c^T W_c   (32x32, accumulated on the TensorEngine in fp32 PSUM)
The device returns prod = Gram .* (blockdiag A / N) as a [128, 256] fp32
matrix; the host sums the 8 matrices (the block-diagonal mask zeroes the
cross-ray garbage, so a plain elementwise sum is the loss).

The kernel computes in fp8e4 (per-element rounding noise averages out over
66M elements: measured rel err ~2e-4, far inside the 2e-2 gate), so
streaming f32 from HBM would be 4x excess traffic.  The host rounds ws once
and stages narrow shards.

Per-core kernel (data parallel over rays, per the sharding hint; raw bass —
hand-rolled semaphores, TileContext's fixed preamble/epilogue is ~18us
here).  Trace-derived design points (ntff on this chip):
  - whole fp8 shard fits in SBUF -> persistent per-tile buffers, no slot
    reuse, no cast stage.  Two HWDGE rings (sync + scalar engines) carry
    alternating tiles; measured aggregate ~360-415 GB/s sustained
  - fp8 DoubleRow matmuls: one MM eats a [128, 2, 128] window (2 planes x
    4 rays x 32 bins per partition), psum += X0^T X0 + X1^T X1.  Off-
    diagonal cross-ray blocks are garbage, masked by the block-diagonal
    weight const in the final elementwise mul.  Warm DR window = ~78ns /
    1024 rays (~420 GB/s) vs ~56ns / 512 rays for the normal 128-col
    window -> ~1.9x PE stream rate; the PE was the sole bottleneck
  - HAM clock-gates an idle PE to 1.2GHz and un-throttles only after
    ~4-6us of gap-free matmul work (any DMA-wait gap resets the timer):
    a warm-up burst on a zeroed scratch buffer bridges the preamble idle,
    and tiles strictly alternate rings so neither queue ever serializes
    enough consecutive tiles to starve the PE (observed 2.8us stall ->
    re-throttle when five early tiles rode one ring)
  - the tail (leftover 1152 rays as a [32-part, 1152-col] tile) loads
    early on the sync ring but is consumed last
  - endgame: the last tile + tail accumulate into a second PSUM half so
    the main Gram closes early; its DVE mul with the mask const overlaps
    the stream tail (the mul's PSUM-visibility fence is a sem inc carried
    by a real matmul >=2 MMs later -- MMs complete strictly in pc order).
    The [128, 256] fp32 prod matrix is DMA'd out whole; the host does the
    final sum.  This removes the old reduce -> fp32 cross-partition
    matmul -> copy -> 4B store chain (~1.8us of serial sem hops)
  - no epilogue sem clears: the NEFF teardown emitted by the compiler
    already zeroes every engine's semaphore range; the entry clears
    (pre-stream, overlapped with DMA spin-up) handle stale state
"""

import numpy as np

NEAR = 0.2
FAR = 1000.0
BINS = 32
N_RAYS = 2073600
N_CORES = 8
N_SHARD = N_RAYS // N_CORES        # 259200 rays per core
P = 128

# "bf16" or "fp8" (float8e4 / e4m3 on device, host-rounded via ml_dtypes)
DTYPE = "fp8"
# fp8 DoubleRow perf mode (see module docstring)
DOUBLE_ROW = True
# N=512 warm-up burst bridging preamble -> first tile (HAM + a PE-idle
# hang observed when the tensor program begins directly with a DMA-sem
# wait).  ~427ns each at the cold clock.  Kept short: a burst MM is pure
# overhead while a COLD real window (136ns for 78ns of work) still
# advances the stream -- the PE must catch the DMA stream as early as
# possible since its warm burn (~420 GB/s) exceeds delivery (~350).
WARM_MMS = 12

# rays-per-partition per main tile; each K divisible by 8 (whole DoubleRow
# windows).  Small first tiles arrive just-in-time for the cold PE; the
# middle stays <=128K rays (<=0.52MB, ~1.5us of delivery) because the
# tracking PE stalls at each tile boundary for the remainder of that
# tile's delivery -- stalls must stay under HAM's ~2us re-throttle
# window.  The last tiles taper so the final data lands with minimal
# latency.
TILE_KS = [24, 32, 48, 64, 96, 152, 152] + [128] * 9 + [104, 80, 56, 32,
           24]
# Tiles alternate rings so the prefix-balance holds at every point of the
# stream (a sync-heavy early ramp measured 9us SLOWER: the sync queue's
# mid-stream tiles then land late).  Slight sync bias in the totals to
# match measured contended rates (scalar/Act ~150 B/ns, sync/SP ~180).
# Each ring's list ascending = its queue order = consumption order.  (A
# third stream via the gpsimd SWDGE was tried and measured 4us SLOWER --
# SWDGE descriptor processing steals the same DMA-engine pool and
# delivers late.)
SCALAR_TILES = (0, 2, 4, 6, 8, 10, 12, 14, 16, 20)
SYNC_TILES = (1, 3, 5, 7, 9, 11, 13, 15, 17, 18, 19)
assert sum(TILE_KS) == 2016
assert all(k % 8 == 0 for k in TILE_KS)
# leftover 1152 rays -> one [32-partition, TAIL_K*32-col] tile (DoubleRow
# windows of 2 x 4 rays x 32 bins, contraction 32, plus one normal window)
TAIL_K = 36
assert sum(TILE_KS) * P + TAIL_K * 32 == N_SHARD
assert TAIL_K % 4 == 0

# set by test.py to capture a neuron-profile trace; harness leaves it False
TRACE = False
TRACE_TMPDIR = None
TRACE_CORES = None
LAST_RESULTS = None


def _a_matrix() -> np.ndarray:
    eps = float(np.finfo(np.float32).eps)
    t = np.linspace(NEAR + eps, FAR, BINS + 1, dtype=np.float32)
    s = ((1.0 / t) - (1.0 / (NEAR + eps))) / ((1.0 / FAR) - (1.0 / (NEAR + eps)))
    s = s.astype(np.float32)
    us = ((s[1:] + s[:-1]) * 0.5).astype(np.float32)
    dus = np.abs(us[:, None] - us[None, :]).astype(np.float32)
    ds = (s[1:] - s[:-1]).astype(np.float32)
    return (dus + np.diag(ds) / 3.0).astype(np.float32)


def _bigw_np() -> np.ndarray:
    a = _a_matrix() / np.float32(N_RAYS)
    bigw = np.zeros((P, P), np.float32)
    for q in range(4):
        bigw[32 * q:32 * q + 32, 32 * q:32 * q + 32] = a
    return bigw


_COMPILED = None


def _build():
    """Two HWDGE rings stream the fp8 shard into persistent SBUF buffers
    while the PE chases them with DoubleRow Gram matmuls.

    sync   : bigw const, tail tile, odd-index tiles (ring B)
    scalar : even-index tiles (ring A), prod store
    vector : warm-up scratch memset, the two masked-product muls
    tensor : warm-up burst, Gram matmuls, fence MMs
    """
    import concourse.bass as bass
    import concourse.mybir as mybir
    from contextlib import ExitStack

    # The Bass constructor unconditionally emits 4 gpsimd memsets for its
    # const-AP pool, then an all-engine barrier — ~3-4us of startup for
    # constants no instruction here reads.  Skip the memsets; keep the
    # barrier.
    _real_memset = bass.BassGpSimd.memset
    bass.BassGpSimd.memset = lambda self, ap, c: None
    try:
        nc = bass.Bass("TRN2", debug=False, enable_partition_id=False)
    finally:
        bass.BassGpSimd.memset = _real_memset
    f32 = mybir.dt.float32
    wdt = mybir.dt.bfloat16 if DTYPE == "bf16" else mybir.dt.float8e4

    bf16 = mybir.dt.bfloat16
    ws = nc.dram_tensor("ws", [N_SHARD, BINS], wdt, kind="ExternalInput")
    # prod is summed (2048 nonzero terms) on the host: bf16 rounding is
    # ~0.4% per element, unbiased -> ~1e-4 on the total, well inside the
    # error budget; halves the store
    out = nc.dram_tensor("out", [P, 2 * P], bf16, kind="ExternalOutput")
    bigw2 = np.concatenate([_bigw_np(), _bigw_np()], axis=1)
    bigw_d = nc.inline_tensor(bigw2, name="bigw")

    T = len(TILE_KS)
    assert sorted(SCALAR_TILES + SYNC_TILES) == list(range(T))

    views = []
    ray0 = 0
    for kt in TILE_KS:
        views.append(
            ws[ray0:ray0 + P * kt, :].rearrange("(p k) b -> p (k b)", p=P, k=kt)
        )
        ray0 += P * kt
    tail_view = ws[ray0:N_SHARD, :].rearrange(
        "(p k) b -> p (k b)", p=32, k=TAIL_K
    )

    bslots = [
        nc.alloc_sbuf_tensor(f"bs{i}", [P, kt * BINS], wdt)
        for i, kt in enumerate(TILE_KS)
    ]
    tail_s = nc.alloc_sbuf_tensor("tail_s", [32, TAIL_K * BINS], wdt)
    warm_s = nc.alloc_sbuf_tensor("warm_s", [P, 512], wdt)
    bigw_s = nc.alloc_sbuf_tensor("bigw_s", [P, 2 * P], f32)
    prod_s = nc.alloc_sbuf_tensor("prod_s", [P, 2 * P], bf16)

    # separate PSUM tensors -> separate banks: the DVE reads the closed
    # main Gram while the PE still accumulates gram2 (concurrent DVE-read +
    # PE-write to the SAME bank error-aborts the NEFF; observed, 3 runs)
    gram_ps = nc.alloc_psum_tensor("gram_ps", [P, P], f32)
    gram2_ps = nc.alloc_psum_tensor("gram2_ps", [P, P], f32)
    warm_ps = nc.alloc_psum_tensor("warm_ps", [P, 512], f32)

    with ExitStack() as ctx:
        # one completion sem PER TILE: the 16 DMA engines interleave
        # completions of consecutive DMAs on the same queue, so a shared
        # ring sem with ">= 16*(i+1)" thresholds can pass while tile i is
        # still in flight (observed: NaN Gram from reading unwritten SBUF)
        sem_tile = [
            ctx.enter_context(nc.semaphore(f"sem_t{i}")) for i in range(T)
        ]
        sem_tail = ctx.enter_context(nc.semaphore("sem_tail"))
        sem_const = ctx.enter_context(nc.semaphore("sem_const"))
        sem_warm = ctx.enter_context(nc.semaphore("sem_warm"))
        sem_pe1 = ctx.enter_context(nc.semaphore("sem_pe1"))
        sem_pe2 = ctx.enter_context(nc.semaphore("sem_pe2"))
        sem_prod = ctx.enter_context(nc.semaphore("sem_prod"))
        sem_out_dma = ctx.enter_context(nc.semaphore("sem_out_dma"))
        # sem_warm is cleared FIRST: the vector engine's warm memset incs it
        # ~300ns into the body, so its clear must land within the first few
        # entry clears (~20ns each) to stay ordered before the inc
        all_sems = [sem_warm] + sem_tile + [
            sem_tail, sem_const, sem_pe1, sem_pe2, sem_prod, sem_out_dma,
        ]

        # Clear every semaphore BEFORE any engine can observe them: other
        # NEFFs (e.g. the jax helpers run on these cores by the caller's
        # process) share the physical semaphore file and can leave nonzero
        # values, which would pre-satisfy the waits below and let engines
        # read SBUF before the DMAs land.  The clears ride the otherwise
        # idle GPSIMD engine (on sync they delayed that ring's first DMA
        # ~0.5us) and finish ~0.5us into the body; the earliest
        # cross-engine observation (the warm memset's inc at ~300ns for
        # sem_warm, cleared first / a DMA completion inc at >2us for the
        # rest) stays ordered after.
        for s in all_sems:
            nc.gpsimd.sem_clear(s)

        with nc.Block() as block:

            @block.sync
            def _(sync):
                # the early ramp leads; bigw + tail are not needed until
                # the endgame, so they ride behind it
                for t in SYNC_TILES[:3]:
                    sync.dma_start(bslots[t][:], views[t]).then_inc(
                        sem_tile[t], 16
                    )
                sync.dma_start(bigw_s[:], bigw_d[:]).then_inc(sem_const, 16)
                sync.dma_start(tail_s[:], tail_view).then_inc(sem_tail, 16)
                for t in SYNC_TILES[3:]:
                    sync.dma_start(bslots[t][:], views[t]).then_inc(
                        sem_tile[t], 16
                    )

            @block.scalar
            def _(scalar):
                for t in SCALAR_TILES:
                    scalar.dma_start(bslots[t][:], views[t]).then_inc(
                        sem_tile[t], 16
                    )
                # prod store in halves: the first half is ready mid-stream
                # (right after the main Gram closes), so only the second
                # 64KB ride the endgame.  The Block-exit DRAIN fences
                # completion (the incs are framework-required, no waiter).
                scalar.wait_ge(sem_prod, 1)
                scalar.dma_start(out[:, 0:P], prod_s[:, 0:P]).then_inc(
                    sem_out_dma, 16
                )
                scalar.wait_ge(sem_prod, 2)
                scalar.dma_start(out[:, P:2 * P], prod_s[:, P:2 * P]).then_inc(
                    sem_out_dma, 16
                )

            @block.vector
            def _(vector):
                vector.memset(warm_s[:], 0.0).then_inc(sem_warm, 1)
                # masked products: prod = gram .* bigw, per PSUM half.
                # The first half runs while the PE still streams the
                # gram2 windows; the second right after the last fence.
                # sem_pe1 first: it can only fire mid-stream, long after
                # the sync engine's entry clears have landed, so the
                # sem_const wait that follows can never pass on a stale
                # pre-clear value.
                vector.wait_ge(sem_pe1, 1)
                vector.wait_ge(sem_const, 16)
                vector.tensor_mul(
                    prod_s[:, 0:P], gram_ps[:], bigw_s[:, 0:P]
                ).then_inc(sem_prod, 1)
                vector.wait_ge(sem_pe2, 1)
                vector.tensor_mul(
                    prod_s[:, P:2 * P], gram2_ps[:], bigw_s[:, P:2 * P]
                ).then_inc(sem_prod, 1)

            @block.tensor
            def _(tensor):
                DR = mybir.MatmulPerfMode.DoubleRow if DOUBLE_ROW else None

                def drw(ap2d):
                    # [p, 256] window -> [p, 2, 128] DoubleRow operand
                    return ap2d.rearrange("p (two f) -> p two f", two=2)

                def gram_mms(dst, src, kt, start0, stop_last, col0=0):
                    """Emit the Gram matmuls for kt rays/partition of tile
                    `src` starting at column col0 (kt % 4 == 0).  DoubleRow
                    eats 8 rays per partition per MM; a 4-ray remainder
                    gets a normal 128-col window."""
                    insts = []
                    if DOUBLE_ROW:
                        n_dr, rem = kt // 8, kt % 8
                    else:
                        n_dr, rem = 0, kt
                    n_win = n_dr + rem // 4
                    w = n_dr * 2
                    for i in range(n_dr):
                        win = src[:, col0 + i * 256:col0 + (i + 1) * 256]
                        insts.append(nc.tensor.matmul(
                            dst,
                            drw(win), drw(win),
                            start=(start0 and i == 0),
                            stop=(stop_last and i == n_win - 1),
                            perf_mode=DR,
                        ))
                    for j in range(rem // 4):
                        win = src[:, col0 + (w + j) * 128:col0 + (w + j + 1) * 128]
                        insts.append(nc.tensor.matmul(
                            dst,
                            win, win,
                            start=(start0 and n_dr == 0 and j == 0),
                            stop=(stop_last and n_dr + j == n_win - 1),
                        ))
                    return insts

                tensor.wait_ge(sem_warm, 1)
                for _ in range(WARM_MMS):
                    nc.tensor.matmul(
                        warm_ps[:], warm_s[:, 0:128], warm_s[:],
                        start=True, stop=True,
                    )
                # main stream: tiles 0..T-2 -> gram_ps
                for t in range(T - 1):
                    tensor.wait_ge(sem_tile[t], 16)
                    gram_mms(
                        gram_ps[:], bslots[t], TILE_KS[t],
                        start0=(t == 0), stop_last=(t == T - 2),
                    )
                # last tile + 32-partition tail tile -> gram2_ps.
                # A matmul's then_inc / an engine drain can fire before its
                # systolic write-back lands in PSUM (observed: torn/partial
                # reads on the DVE).  MMs complete strictly in pc order, so
                # a sem inc attached >= 2 matmuls later is a sound PSUM
                # fence: a cheap N=64 warm matmul inserted after two gram2
                # windows fences the whole main Gram for ~30ns of stream.
                tensor.wait_ge(sem_tile[T - 1], 16)
                assert TILE_KS[T - 1] >= 24
                gram_mms(
                    gram2_ps[:], bslots[T - 1], 16,
                    start0=True, stop_last=False,
                )
                nc.tensor.matmul(
                    warm_ps[:, 0:64], warm_s[:, 0:128], warm_s[:, 0:64],
                    start=True, stop=True,
                ).then_inc(sem_pe1, 1)
                gram_mms(
                    gram2_ps[:], bslots[T - 1], TILE_KS[T - 1] - 16,
                    start0=False, stop_last=False, col0=16 * BINS,
                )
                tensor.wait_ge(sem_tail, 16)
                gram_mms(
                    gram2_ps[:], tail_s, TAIL_K,
                    start0=False, stop_last=True,
                )
                # fence for gram2: two small MMs, the second carries the inc
                nc.tensor.matmul(
                    warm_ps[:, 0:64], warm_s[:, 0:128], warm_s[:, 0:64],
                    start=True, stop=True,
                )
                nc.tensor.matmul(
                    warm_ps[:, 0:64], warm_s[:, 0:128], warm_s[:, 0:64],
                    start=True, stop=True,
                ).then_inc(sem_pe2, 1)

        # No explicit receipt wait for the prod store: the scalar engine's
        # Block-exit DRAIN fences its HWDGE queue (outstanding DMAs must
        # retire before the drain completes), which saves the ~0.9us
        # completion-semaphore propagation on the critical path.  The
        # compiler-emitted NEFF teardown clears every semaphore, so no
        # explicit epilogue clears are needed either.

    return nc


def kernel(ws: np.ndarray) -> np.ndarray:
    import ml_dtypes
    from concourse.bass_utils import run_bass_kernel_spmd

    global _COMPILED, LAST_RESULTS
    if _COMPILED is None:
        _COMPILED = _build()
    nc = _COMPILED

    ws = np.asarray(ws)
    assert ws.shape == (N_RAYS, BINS), ws.shape
    # round once on the host: the device computes in this dtype anyway, and
    # streaming f32 from HBM would be excess traffic
    hdt = ml_dtypes.bfloat16 if DTYPE == "bf16" else ml_dtypes.float8_e4m3
    wsq = np.ascontiguousarray(ws).astype(hdt)
    shards = wsq.reshape(N_CORES, N_SHARD, BINS)
    in_maps = [{"ws": shards[c]} for c in range(N_CORES)]
    res = run_bass_kernel_spmd(
        nc, in_maps, list(range(N_CORES)), trace=TRACE, tmpdir=TRACE_TMPDIR,
        trace_cores=TRACE_CORES,
    )
    LAST_RESULTS = res
    total = np.float64(0.0)
    for c in range(N_CORES):
        total += np.sum(res.results[c]["out"].astype(np.float64))
    return np.array(total, dtype=np.float32)


# revision 43
# speedup vs baseline: 1.0202x; 1.0202x over previous
"""Distortion-regularization loss on Trainium2 (8 NeuronCores, SPMD).

Math: the reference loss collapses to a single quadratic form
    loss = mean_n( w_n^T A w_n ),   A = |u_i - u_j| + diag(ds)/3   (32x32 const)
         = <A, W^T W> / N_RAYS
so each core only needs the Gram matrix of its ray shard:
    Gram_c = W_c^T W_c   (32x32, accumulated on the TensorEngine in fp32 PSUM)
The device returns prod = Gram .* (blockdiag A / N) as a [128, 256] fp32
matrix; the host sums the 8 matrices (the block-diagonal mask zeroes the
cross-ray garbage, so a plain elementwise sum is the loss).

The kernel computes in fp8e4 (per-element rounding noise averages out over
66M elements: measured rel err ~2e-4, far inside the 2e-2 gate), so
streaming f32 from HBM would be 4x excess traffic.  The host rounds ws once
and stages narrow shards.

Per-core kernel (data parallel over rays, per the sharding hint; raw bass —
hand-rolled semaphores, TileContext's fixed preamble/epilogue is ~18us
here).  Trace-derived design points (ntff on this chip):
  - whole fp8 shard fits in SBUF -> persistent per-tile buffers, no slot
    reuse, no cast stage.  Two HWDGE rings (sync + scalar engines) carry
    alternating tiles; measured aggregate ~360-415 GB/s sustained
  - fp8 DoubleRow matmuls: one MM eats a [128, 2, 128] window (2 planes x
    4 rays x 32 bins per partition), psum += X0^T X0 + X1^T X1.  Off-
    diagonal cross-ray blocks are garbage, masked by the block-diagonal
    weight const in the final elementwise mul.  Warm DR window = ~78ns /
    1024 rays (~420 GB/s) vs ~56ns / 512 rays for the normal 128-col
    window -> ~1.9x PE stream rate; the PE was the sole bottleneck
  - HAM clock-gates an idle PE to 1.2GHz and un-throttles only after
    ~4-6us of gap-free matmul work (any DMA-wait gap resets the timer):
    a warm-up burst on a zeroed scratch buffer bridges the preamble idle,
    and tiles strictly alternate rings so neither queue ever serializes
    enough consecutive tiles to starve the PE (observed 2.8us stall ->
    re-throttle when five early tiles rode one ring)
  - the tail (leftover 1152 rays as a [32-part, 1152-col] tile) loads
    early on the sync ring but is consumed last
  - endgame: the last tile + tail accumulate into a second PSUM half so
    the main Gram closes early; its DVE mul with the mask const overlaps
    the stream tail (the mul's PSUM-visibility fence is a sem inc carried
    by a real matmul >=2 MMs later -- MMs complete strictly in pc order).
    The [128, 256] fp32 prod matrix is DMA'd out whole; the host does the
    final sum.  This removes the old reduce -> fp32 cross-partition
    matmul -> copy -> 4B store chain (~1.8us of serial sem hops)
  - no epilogue sem clears: the NEFF teardown emitted by the compiler
    already zeroes every engine's semaphore range; the entry clears
    (pre-stream, overlapped with DMA spin-up) handle stale state
"""

import numpy as np

NEAR = 0.2
FAR = 1000.0
BINS = 32
N_RAYS = 2073600
N_CORES = 8
N_SHARD = N_RAYS // N_CORES        # 259200 rays per core
P = 128

# "bf16" or "fp8" (float8e4 / e4m3 on device, host-rounded via ml_dtypes)
DTYPE = "fp8"
# fp8 DoubleRow perf mode (see module docstring)
DOUBLE_ROW = True
# N=512 warm-up burst bridging preamble -> first tile (HAM + a PE-idle
# hang observed when the tensor program begins directly with a DMA-sem
# wait).  ~427ns each at the cold clock.  Kept short: a burst MM is pure
# overhead while a COLD real window (136ns for 78ns of work) still
# advances the stream -- the PE must catch the DMA stream as early as
# possible since its warm burn (~420 GB/s) exceeds delivery (~350).
WARM_MMS = 7

# rays-per-partition per main tile; each K divisible by 8 (whole DoubleRow
# windows).  Small first tiles arrive just-in-time for the cold PE; the
# middle stays <=128K rays (<=0.52MB, ~1.5us of delivery) because the
# tracking PE stalls at each tile boundary for the remainder of that
# tile's delivery -- stalls must stay under HAM's ~2us re-throttle
# window.  The last tiles taper so the final data lands with minimal
# latency.
TILE_KS = [24, 32, 48, 64, 96, 96, 112, 112, 112] + [128] * 8 + [104, 80,
           56, 32, 24]
# Tiles alternate rings so the prefix-balance holds at every point of the
# stream (a sync-heavy early ramp measured 9us SLOWER: the sync queue's
# mid-stream tiles then land late).  Slight sync bias in the totals to
# match measured contended rates (scalar/Act ~150 B/ns, sync/SP ~180).
# Each ring's list ascending = its queue order = consumption order.  (A
# third stream via the gpsimd SWDGE was tried and measured 4us SLOWER --
# SWDGE descriptor processing steals the same DMA-engine pool and
# delivers late.)
SCALAR_TILES = (0, 2, 4, 6, 8, 10, 12, 14, 16, 18)
SYNC_TILES = (1, 3, 5, 7, 9, 11, 13, 15, 17, 19, 20, 21)
assert sum(TILE_KS) == 2016
assert all(k % 8 == 0 for k in TILE_KS)
# leftover 1152 rays -> one [32-partition, TAIL_K*32-col] tile (DoubleRow
# windows of 2 x 4 rays x 32 bins, contraction 32, plus one normal window)
TAIL_K = 36
assert sum(TILE_KS) * P + TAIL_K * 32 == N_SHARD
assert TAIL_K % 4 == 0

# set by test.py to capture a neuron-profile trace; harness leaves it False
TRACE = False
TRACE_TMPDIR = None
TRACE_CORES = None
LAST_RESULTS = None


def _a_matrix() -> np.ndarray:
    eps = float(np.finfo(np.float32).eps)
    t = np.linspace(NEAR + eps, FAR, BINS + 1, dtype=np.float32)
    s = ((1.0 / t) - (1.0 / (NEAR + eps))) / ((1.0 / FAR) - (1.0 / (NEAR + eps)))
    s = s.astype(np.float32)
    us = ((s[1:] + s[:-1]) * 0.5).astype(np.float32)
    dus = np.abs(us[:, None] - us[None, :]).astype(np.float32)
    ds = (s[1:] - s[:-1]).astype(np.float32)
    return (dus + np.diag(ds) / 3.0).astype(np.float32)


def _bigw_np() -> np.ndarray:
    a = _a_matrix() / np.float32(N_RAYS)
    bigw = np.zeros((P, P), np.float32)
    for q in range(4):
        bigw[32 * q:32 * q + 32, 32 * q:32 * q + 32] = a
    return bigw


_COMPILED = None


def _build():
    """Two HWDGE rings stream the fp8 shard into persistent SBUF buffers
    while the PE chases them with DoubleRow Gram matmuls.

    sync   : bigw const, tail tile, odd-index tiles (ring B)
    scalar : even-index tiles (ring A), prod store
    vector : warm-up scratch memset, the two masked-product muls
    tensor : warm-up burst, Gram matmuls, fence MMs
    """
    import concourse.bass as bass
    import concourse.mybir as mybir
    from contextlib import ExitStack

    # The Bass constructor unconditionally emits 4 gpsimd memsets for its
    # const-AP pool, then an all-engine barrier — ~3-4us of startup for
    # constants no instruction here reads.  Skip the memsets; keep the
    # barrier.
    _real_memset = bass.BassGpSimd.memset
    bass.BassGpSimd.memset = lambda self, ap, c: None
    try:
        nc = bass.Bass("TRN2", debug=False, enable_partition_id=False)
    finally:
        bass.BassGpSimd.memset = _real_memset
    # the Pool (gpsimd SWDGE) dynamic queue is never used; dropping its
    # declaration trims the runtime's per-queue NEFF-entry init
    nc.m.queues[:] = [q for q in nc.m.queues if q.name != "qPoolDynamic"]
    f32 = mybir.dt.float32
    wdt = mybir.dt.bfloat16 if DTYPE == "bf16" else mybir.dt.float8e4

    bf16 = mybir.dt.bfloat16
    ws = nc.dram_tensor("ws", [N_SHARD, BINS], wdt, kind="ExternalInput")
    # prod is summed (2048 nonzero terms) on the host: bf16 rounding is
    # ~0.4% per element, unbiased -> ~1e-4 on the total, well inside the
    # error budget; halves the store
    out = nc.dram_tensor("out", [P, 2 * P], bf16, kind="ExternalOutput")
    bigw2 = np.concatenate([_bigw_np(), _bigw_np()], axis=1)
    bigw_d = nc.inline_tensor(bigw2, name="bigw")

    T = len(TILE_KS)
    assert sorted(SCALAR_TILES + SYNC_TILES) == list(range(T))

    views = []
    ray0 = 0
    for kt in TILE_KS:
        views.append(
            ws[ray0:ray0 + P * kt, :].rearrange("(p k) b -> p (k b)", p=P, k=kt)
        )
        ray0 += P * kt
    tail_view = ws[ray0:N_SHARD, :].rearrange(
        "(p k) b -> p (k b)", p=32, k=TAIL_K
    )

    bslots = [
        nc.alloc_sbuf_tensor(f"bs{i}", [P, kt * BINS], wdt)
        for i, kt in enumerate(TILE_KS)
    ]
    tail_s = nc.alloc_sbuf_tensor("tail_s", [32, TAIL_K * BINS], wdt)
    warm_s = nc.alloc_sbuf_tensor("warm_s", [P, 512], wdt)
    bigw_s = nc.alloc_sbuf_tensor("bigw_s", [P, 2 * P], f32)
    prod_s = nc.alloc_sbuf_tensor("prod_s", [P, 2 * P], bf16)

    # separate PSUM tensors -> separate banks: the DVE reads the closed
    # main Gram while the PE still accumulates gram2 (concurrent DVE-read +
    # PE-write to the SAME bank error-aborts the NEFF; observed, 3 runs)
    gram_ps = nc.alloc_psum_tensor("gram_ps", [P, P], f32)
    gram2_ps = nc.alloc_psum_tensor("gram2_ps", [P, P], f32)
    warm_ps = nc.alloc_psum_tensor("warm_ps", [P, 512], f32)

    with ExitStack() as ctx:
        # one completion sem PER TILE: the 16 DMA engines interleave
        # completions of consecutive DMAs on the same queue, so a shared
        # ring sem with ">= 16*(i+1)" thresholds can pass while tile i is
        # still in flight (observed: NaN Gram from reading unwritten SBUF)
        sem_tile = [
            ctx.enter_context(nc.semaphore(f"sem_t{i}")) for i in range(T)
        ]
        sem_tail = ctx.enter_context(nc.semaphore("sem_tail"))
        sem_const = ctx.enter_context(nc.semaphore("sem_const"))
        sem_warm = ctx.enter_context(nc.semaphore("sem_warm"))
        sem_pe1 = ctx.enter_context(nc.semaphore("sem_pe1"))
        sem_pe2 = ctx.enter_context(nc.semaphore("sem_pe2"))
        sem_prod = ctx.enter_context(nc.semaphore("sem_prod"))
        sem_out_dma = ctx.enter_context(nc.semaphore("sem_out_dma"))
        # sem_warm is cleared FIRST: the vector engine's warm memset incs it
        # ~300ns into the body, so its clear must land within the first few
        # entry clears (~20ns each) to stay ordered before the inc
        all_sems = [sem_warm] + sem_tile + [
            sem_tail, sem_const, sem_pe1, sem_pe2, sem_prod, sem_out_dma,
        ]

        # Clear every semaphore BEFORE any engine can observe them: other
        # NEFFs (e.g. the jax helpers run on these cores by the caller's
        # process) share the physical semaphore file and can leave nonzero
        # values, which would pre-satisfy the waits below and let engines
        # read SBUF before the DMAs land.  The clears ride the otherwise
        # idle GPSIMD engine (on sync they delayed that ring's first DMA
        # ~0.5us) and finish ~0.5us into the body; the earliest
        # cross-engine observation (the warm memset's inc at ~300ns for
        # sem_warm, cleared first / a DMA completion inc at >2us for the
        # rest) stays ordered after.
        for s in all_sems:
            nc.gpsimd.sem_clear(s)

        with nc.Block(no_gpsimd_drain=True) as block:

            @block.sync
            def _(sync):
                # the early ramp leads; bigw + tail are not needed until
                # the endgame, so they ride behind it
                for t in SYNC_TILES[:3]:
                    sync.dma_start(bslots[t][:], views[t]).then_inc(
                        sem_tile[t], 16
                    )
                sync.dma_start(bigw_s[:], bigw_d[:]).then_inc(sem_const, 16)
                sync.dma_start(tail_s[:], tail_view).then_inc(sem_tail, 16)
                for t in SYNC_TILES[3:]:
                    sync.dma_start(bslots[t][:], views[t]).then_inc(
                        sem_tile[t], 16
                    )

            @block.scalar
            def _(scalar):
                for t in SCALAR_TILES:
                    scalar.dma_start(bslots[t][:], views[t]).then_inc(
                        sem_tile[t], 16
                    )
                # prod store in halves: the first half is ready mid-stream
                # (right after the main Gram closes), so only the second
                # 64KB ride the endgame.  The Block-exit DRAIN fences
                # completion (the incs are framework-required, no waiter).
                scalar.wait_ge(sem_prod, 1)
                scalar.dma_start(out[:, 0:P], prod_s[:, 0:P]).then_inc(
                    sem_out_dma, 16
                )
                scalar.wait_ge(sem_prod, 2)
                scalar.dma_start(out[:, P:2 * P], prod_s[:, P:2 * P]).then_inc(
                    sem_out_dma, 16
                )

            @block.vector
            def _(vector):
                vector.memset(warm_s[:], 0.0).then_inc(sem_warm, 1)
                # masked products: prod = gram .* bigw, per PSUM half.
                # The first half runs while the PE still streams the
                # gram2 windows; the second right after the last fence.
                # sem_pe1 first: it can only fire mid-stream, long after
                # the sync engine's entry clears have landed, so the
                # sem_const wait that follows can never pass on a stale
                # pre-clear value.
                vector.wait_ge(sem_pe1, 1)
                vector.wait_ge(sem_const, 16)
                vector.tensor_mul(
                    prod_s[:, 0:P], gram_ps[:], bigw_s[:, 0:P]
                ).then_inc(sem_prod, 1)
                vector.wait_ge(sem_pe2, 1)
                vector.tensor_mul(
                    prod_s[:, P:2 * P], gram2_ps[:], bigw_s[:, P:2 * P]
                ).then_inc(sem_prod, 1)

            @block.tensor
            def _(tensor):
                DR = mybir.MatmulPerfMode.DoubleRow if DOUBLE_ROW else None

                def drw(ap2d):
                    # [p, 256] window -> [p, 2, 128] DoubleRow operand
                    return ap2d.rearrange("p (two f) -> p two f", two=2)

                def gram_mms(dst, src, kt, start0, stop_last, col0=0):
                    """Emit the Gram matmuls for kt rays/partition of tile
                    `src` starting at column col0 (kt % 4 == 0).  DoubleRow
                    eats 8 rays per partition per MM; a 4-ray remainder
                    gets a normal 128-col window."""
                    insts = []
                    if DOUBLE_ROW:
                        n_dr, rem = kt // 8, kt % 8
                    else:
                        n_dr, rem = 0, kt
                    n_win = n_dr + rem // 4
                    w = n_dr * 2
                    for i in range(n_dr):
                        win = src[:, col0 + i * 256:col0 + (i + 1) * 256]
                        insts.append(nc.tensor.matmul(
                            dst,
                            drw(win), drw(win),
                            start=(start0 and i == 0),
                            stop=(stop_last and i == n_win - 1),
                            perf_mode=DR,
                        ))
                    for j in range(rem // 4):
                        win = src[:, col0 + (w + j) * 128:col0 + (w + j + 1) * 128]
                        insts.append(nc.tensor.matmul(
                            dst,
                            win, win,
                            start=(start0 and n_dr == 0 and j == 0),
                            stop=(stop_last and n_dr + j == n_win - 1),
                        ))
                    return insts

                tensor.wait_ge(sem_warm, 1)
                for _ in range(WARM_MMS):
                    nc.tensor.matmul(
                        warm_ps[:], warm_s[:, 0:128], warm_s[:],
                        start=True, stop=True,
                    )
                # main stream: tiles 0..T-2 -> gram_ps
                for t in range(T - 1):
                    tensor.wait_ge(sem_tile[t], 16)
                    gram_mms(
                        gram_ps[:], bslots[t], TILE_KS[t],
                        start0=(t == 0), stop_last=(t == T - 2),
                    )
                # last tile + 32-partition tail tile -> gram2_ps.
                # A matmul's then_inc / an engine drain can fire before its
                # systolic write-back lands in PSUM (observed: torn/partial
                # reads on the DVE).  MMs complete strictly in pc order, so
                # a sem inc attached >= 2 matmuls later is a sound PSUM
                # fence: a cheap N=64 warm matmul inserted after two gram2
                # windows fences the whole main Gram for ~30ns of stream.
                tensor.wait_ge(sem_tile[T - 1], 16)
                assert TILE_KS[T - 1] >= 24
                gram_mms(
                    gram2_ps[:], bslots[T - 1], 16,
                    start0=True, stop_last=False,
                )
                nc.tensor.matmul(
                    warm_ps[:, 0:64], warm_s[:, 0:128], warm_s[:, 0:64],
                    start=True, stop=True,
                ).then_inc(sem_pe1, 1)
                gram_mms(
                    gram2_ps[:], bslots[T - 1], TILE_KS[T - 1] - 16,
                    start0=False, stop_last=False, col0=16 * BINS,
                )
                tensor.wait_ge(sem_tail, 16)
                gram_mms(
                    gram2_ps[:], tail_s, TAIL_K,
                    start0=False, stop_last=True,
                )
                # fence for gram2: two small MMs, the second carries the inc
                nc.tensor.matmul(
                    warm_ps[:, 0:64], warm_s[:, 0:128], warm_s[:, 0:64],
                    start=True, stop=True,
                )
                nc.tensor.matmul(
                    warm_ps[:, 0:64], warm_s[:, 0:128], warm_s[:, 0:64],
                    start=True, stop=True,
                ).then_inc(sem_pe2, 1)

        # No explicit receipt wait for the prod store: the scalar engine's
        # Block-exit DRAIN fences its HWDGE queue (outstanding DMAs must
        # retire before the drain completes), which saves the ~0.9us
        # completion-semaphore propagation on the critical path.  The
        # compiler-emitted NEFF teardown clears every semaphore, so no
        # explicit epilogue clears are needed either.

    return nc


def kernel(ws: np.ndarray) -> np.ndarray:
    import ml_dtypes
    from concourse.bass_utils import run_bass_kernel_spmd

    global _COMPILED, LAST_RESULTS
    if _COMPILED is None:
        _COMPILED = _build()
    nc = _COMPILED

    ws = np.asarray(ws)
    assert ws.shape == (N_RAYS, BINS), ws.shape
    # round once on the host: the device computes in this dtype anyway, and
    # streaming f32 from HBM would be excess traffic
    hdt = ml_dtypes.bfloat16 if DTYPE == "bf16" else ml_dtypes.float8_e4m3
    wsq = np.ascontiguousarray(ws).astype(hdt)
    shards = wsq.reshape(N_CORES, N_SHARD, BINS)
    in_maps = [{"ws": shards[c]} for c in range(N_CORES)]
    res = run_bass_kernel_spmd(
        nc, in_maps, list(range(N_CORES)), trace=TRACE, tmpdir=TRACE_TMPDIR,
        trace_cores=TRACE_CORES,
    )
    LAST_RESULTS = res
    total = np.float64(0.0)
    for c in range(N_CORES):
        total += np.sum(res.results[c]["out"].astype(np.float64))
    return np.array(total, dtype=np.float32)


# revision 44
# speedup vs baseline: 1.1319x; 1.1095x over previous
"""Distortion-regularization loss on Trainium2 (8 NeuronCores, SPMD).

Math: the reference loss collapses to a single quadratic form
    loss = mean_n( w_n^T A w_n ),   A = |u_i - u_j| + diag(ds)/3   (32x32 const)
         = <A, W^T W> / N_RAYS
so each core only needs the Gram matrix of its ray shard:
    Gram_c = W_c^T W_c   (32x32, accumulated on the TensorEngine in fp32 PSUM)
The device returns prod = Gram .* (blockdiag A / N) as a [128, 256] bf16
matrix; the host sums the 8 matrices (the block-diagonal mask zeroes the
cross-ray garbage, so a plain elementwise sum is the loss).

The kernel computes in fp8e4 (per-element rounding noise averages out over
66M elements: measured rel err ~2e-4, far inside the 2e-2 gate), so
streaming f32 from HBM would be 4x excess traffic.  The host rounds ws once
and stages narrow shards.

Per-core kernel (data parallel over rays, per the sharding hint; raw bass —
hand-rolled semaphores, TileContext's fixed preamble/epilogue is ~18us
here).  Trace-derived design points (ntff on this chip):
  - whole fp8 shard fits in SBUF -> persistent per-tile buffers, no slot
    reuse, no cast stage.  Two HWDGE rings (sync + scalar engines) carry
    alternating tiles; measured aggregate ~360-415 GB/s sustained
  - fp8 DoubleRow matmuls: one MM eats a [128, 2, 128] window (2 planes x
    4 rays x 32 bins per partition), psum += X0^T X0 + X1^T X1.  Off-
    diagonal cross-ray blocks are garbage, masked by the block-diagonal
    weight const in the final elementwise mul.  Warm DR window = ~78ns /
    1024 rays (~420 GB/s) vs ~56ns / 512 rays for the normal 128-col
    window -> ~1.9x PE stream rate; the PE was the sole bottleneck
  - HAM clock-gates an idle PE to 1.2GHz and un-throttles only after
    ~4-6us of gap-free matmul work (any DMA-wait gap resets the timer):
    a warm-up burst on a zeroed scratch buffer bridges the preamble idle,
    and tiles strictly alternate rings so neither queue ever serializes
    enough consecutive tiles to starve the PE (observed 2.8us stall ->
    re-throttle when five early tiles rode one ring)
  - the tail (leftover 1152 rays as a [32-part, 1152-col] tile) loads
    early on the sync ring but is consumed last
  - endgame: the last tile + tail accumulate into a second PSUM half so
    the main Gram closes early; its DVE mul with the mask const overlaps
    the stream tail (the mul's PSUM-visibility fence is a sem inc carried
    by a real matmul >=2 MMs later -- MMs complete strictly in pc order).
    The [128, 256] bf16 prod matrix is DMA'd out in two halves (the
    first mid-stream); the host does the final sum.  This removes the
    old reduce -> fp32 cross-partition matmul -> copy -> 4B store chain
    (~1.8us of serial sem hops), and the store's completion receipt is
    fenced by the scalar engine's Block-exit DRAIN instead of an
    explicit ~0.9us semaphore wait
  - no epilogue sem clears: the NEFF teardown emitted by the compiler
    already zeroes every engine's semaphore range; the entry clears
    (pre-stream, overlapped with DMA spin-up) handle stale state
"""

import numpy as np

NEAR = 0.2
FAR = 1000.0
BINS = 32
N_RAYS = 2073600
N_CORES = 8
N_SHARD = N_RAYS // N_CORES        # 259200 rays per core
P = 128

# "bf16" or "fp8" (float8e4 / e4m3 on device, host-rounded via ml_dtypes)
DTYPE = "fp8"
# fp8 DoubleRow perf mode (see module docstring)
DOUBLE_ROW = True
# N=512 warm-up burst bridging preamble -> first tile (HAM + a PE-idle
# hang observed when the tensor program begins directly with a DMA-sem
# wait).  ~427ns each at the cold clock.  Kept short: a burst MM is pure
# overhead while a COLD real window (136ns for 78ns of work) still
# advances the stream -- the PE must catch the DMA stream as early as
# possible since its warm burn (~420 GB/s) exceeds delivery (~350).
WARM_MMS = 7

# rays-per-partition per main tile; each K divisible by 8 (whole DoubleRow
# windows).  Small first tiles arrive just-in-time for the cold PE; the
# middle stays <=128K rays (<=0.52MB, ~1.5us of delivery) because the
# tracking PE stalls at each tile boundary for the remainder of that
# tile's delivery -- stalls must stay under HAM's ~2us re-throttle
# window.  The last tiles taper so the final data lands with minimal
# latency.
TILE_KS = [24, 32, 48, 64, 96, 96, 112, 112, 112] + [128] * 8 + [104, 80,
           56, 32, 24]
# Tiles alternate rings so the prefix-balance holds at every point of the
# stream (a sync-heavy early ramp measured 9us SLOWER: the sync queue's
# mid-stream tiles then land late).  Slight sync bias in the totals to
# match measured contended rates (scalar/Act ~150 B/ns, sync/SP ~180).
# Each ring's list ascending = its queue order = consumption order.  (A
# third stream via the gpsimd SWDGE was tried and measured 4us SLOWER --
# SWDGE descriptor processing steals the same DMA-engine pool and
# delivers late.)
SCALAR_TILES = (0, 2, 4, 6, 8, 10, 12, 14, 16, 18)
SYNC_TILES = (1, 3, 5, 7, 9, 11, 13, 15, 17, 19, 20, 21)
assert sum(TILE_KS) == 2016
assert all(k % 8 == 0 for k in TILE_KS)
# leftover 1152 rays -> one [32-partition, TAIL_K*32-col] tile (DoubleRow
# windows of 2 x 4 rays x 32 bins, contraction 32, plus one normal window)
TAIL_K = 36
assert sum(TILE_KS) * P + TAIL_K * 32 == N_SHARD
assert TAIL_K % 4 == 0

# set by test.py to capture a neuron-profile trace; harness leaves it False
TRACE = False
TRACE_TMPDIR = None
TRACE_CORES = None
LAST_RESULTS = None


def _a_matrix() -> np.ndarray:
    eps = float(np.finfo(np.float32).eps)
    t = np.linspace(NEAR + eps, FAR, BINS + 1, dtype=np.float32)
    s = ((1.0 / t) - (1.0 / (NEAR + eps))) / ((1.0 / FAR) - (1.0 / (NEAR + eps)))
    s = s.astype(np.float32)
    us = ((s[1:] + s[:-1]) * 0.5).astype(np.float32)
    dus = np.abs(us[:, None] - us[None, :]).astype(np.float32)
    ds = (s[1:] - s[:-1]).astype(np.float32)
    return (dus + np.diag(ds) / 3.0).astype(np.float32)


def _bigw_np() -> np.ndarray:
    a = _a_matrix() / np.float32(N_RAYS)
    bigw = np.zeros((P, P), np.float32)
    for q in range(4):
        bigw[32 * q:32 * q + 32, 32 * q:32 * q + 32] = a
    return bigw


_COMPILED = None


def _build():
    """Two HWDGE rings stream the fp8 shard into persistent SBUF buffers
    while the PE chases them with DoubleRow Gram matmuls.

    sync   : even-index tiles + bigw const + tail tile (ring A; SP's
             queue starts ~0.5us earlier and runs ~15% faster, so it
             leads the ramp and carries slightly more bytes)
    scalar : odd-index tiles (ring B), prod store
    vector : warm-up scratch memset, the two masked-product muls
    tensor : warm-up burst, Gram matmuls, fence MMs
    """
    import concourse.bass as bass
    import concourse.mybir as mybir
    from contextlib import ExitStack

    # The Bass constructor unconditionally emits 4 gpsimd memsets for its
    # const-AP pool, then an all-engine barrier — ~3-4us of startup for
    # constants no instruction here reads.  Skip the memsets; keep the
    # barrier.
    _real_memset = bass.BassGpSimd.memset
    bass.BassGpSimd.memset = lambda self, ap, c: None
    try:
        nc = bass.Bass("TRN2", debug=False, enable_partition_id=False)
    finally:
        bass.BassGpSimd.memset = _real_memset
    # the Pool (gpsimd SWDGE) dynamic queue is never used; dropping its
    # declaration trims the runtime's per-queue NEFF-entry init
    nc.m.queues[:] = [q for q in nc.m.queues if q.name != "qPoolDynamic"]
    f32 = mybir.dt.float32
    wdt = mybir.dt.bfloat16 if DTYPE == "bf16" else mybir.dt.float8e4

    bf16 = mybir.dt.bfloat16
    ws = nc.dram_tensor("ws", [N_SHARD, BINS], wdt, kind="ExternalInput")
    # prod is summed (2048 nonzero terms) on the host: bf16 rounding is
    # ~0.4% per element, unbiased -> ~1e-4 on the total, well inside the
    # error budget; halves the store
    out = nc.dram_tensor("out", [P, 2 * P], bf16, kind="ExternalOutput")
    bigw2 = np.concatenate([_bigw_np(), _bigw_np()], axis=1)
    bigw_d = nc.inline_tensor(bigw2, name="bigw")

    T = len(TILE_KS)
    assert sorted(SCALAR_TILES + SYNC_TILES) == list(range(T))

    views = []
    ray0 = 0
    for kt in TILE_KS:
        views.append(
            ws[ray0:ray0 + P * kt, :].rearrange("(p k) b -> p (k b)", p=P, k=kt)
        )
        ray0 += P * kt
    tail_view = ws[ray0:N_SHARD, :].rearrange(
        "(p k) b -> p (k b)", p=32, k=TAIL_K
    )

    bslots = [
        nc.alloc_sbuf_tensor(f"bs{i}", [P, kt * BINS], wdt)
        for i, kt in enumerate(TILE_KS)
    ]
    tail_s = nc.alloc_sbuf_tensor("tail_s", [32, TAIL_K * BINS], wdt)
    warm_s = nc.alloc_sbuf_tensor("warm_s", [P, 512], wdt)
    bigw_s = nc.alloc_sbuf_tensor("bigw_s", [P, 2 * P], f32)
    prod_s = nc.alloc_sbuf_tensor("prod_s", [P, 2 * P], bf16)

    # separate PSUM tensors -> separate banks: the DVE reads the closed
    # main Gram while the PE still accumulates gram2 (concurrent DVE-read +
    # PE-write to the SAME bank error-aborts the NEFF; observed, 3 runs)
    gram_ps = nc.alloc_psum_tensor("gram_ps", [P, P], f32)
    gram2_ps = nc.alloc_psum_tensor("gram2_ps", [P, P], f32)
    warm_ps = nc.alloc_psum_tensor("warm_ps", [P, 512], f32)

    with ExitStack() as ctx:
        # one completion sem PER TILE: the 16 DMA engines interleave
        # completions of consecutive DMAs on the same queue, so a shared
        # ring sem with ">= 16*(i+1)" thresholds can pass while tile i is
        # still in flight (observed: NaN Gram from reading unwritten SBUF)
        sem_tile = [
            ctx.enter_context(nc.semaphore(f"sem_t{i}")) for i in range(T)
        ]
        sem_tail = ctx.enter_context(nc.semaphore("sem_tail"))
        sem_const = ctx.enter_context(nc.semaphore("sem_const"))
        sem_warm = ctx.enter_context(nc.semaphore("sem_warm"))
        sem_pe1 = ctx.enter_context(nc.semaphore("sem_pe1"))
        sem_pe2 = ctx.enter_context(nc.semaphore("sem_pe2"))
        sem_prod = ctx.enter_context(nc.semaphore("sem_prod"))
        sem_out_dma = ctx.enter_context(nc.semaphore("sem_out_dma"))
        # sem_warm is cleared FIRST: the vector engine's warm memset incs it
        # ~300ns into the body, so its clear must land within the first few
        # entry clears (~20ns each) to stay ordered before the inc
        all_sems = [sem_warm] + sem_tile + [
            sem_tail, sem_const, sem_pe1, sem_pe2, sem_prod, sem_out_dma,
        ]

        # Clear every semaphore BEFORE any engine can observe them: other
        # NEFFs (e.g. the jax helpers run on these cores by the caller's
        # process) share the physical semaphore file and can leave nonzero
        # values, which would pre-satisfy the waits below and let engines
        # read SBUF before the DMAs land.  The clears ride the otherwise
        # idle GPSIMD engine (on sync they delayed that ring's first DMA
        # ~0.5us) and finish ~0.5us into the body; the earliest
        # cross-engine observation (the warm memset's inc at ~300ns for
        # sem_warm, cleared first / a DMA completion inc at >2us for the
        # rest) stays ordered after.
        for s in all_sems:
            nc.gpsimd.sem_clear(s)

        with nc.Block(no_gpsimd_drain=True) as block:

            @block.sync
            def _(sync):
                # the early ramp leads; bigw + tail are not needed until
                # the endgame, so they ride behind it
                for t in SYNC_TILES[:3]:
                    sync.dma_start(bslots[t][:], views[t]).then_inc(
                        sem_tile[t], 16
                    )
                sync.dma_start(bigw_s[:], bigw_d[:]).then_inc(sem_const, 16)
                sync.dma_start(tail_s[:], tail_view).then_inc(sem_tail, 16)
                for t in SYNC_TILES[3:]:
                    sync.dma_start(bslots[t][:], views[t]).then_inc(
                        sem_tile[t], 16
                    )

            @block.scalar
            def _(scalar):
                for t in SCALAR_TILES:
                    scalar.dma_start(bslots[t][:], views[t]).then_inc(
                        sem_tile[t], 16
                    )
                # prod store in halves: the first half is ready mid-stream
                # (right after the main Gram closes), so only the second
                # 64KB ride the endgame.  The Block-exit DRAIN fences
                # completion (the incs are framework-required, no waiter).
                scalar.wait_ge(sem_prod, 1)
                scalar.dma_start(out[:, 0:P], prod_s[:, 0:P]).then_inc(
                    sem_out_dma, 16
                )
                scalar.wait_ge(sem_prod, 2)
                scalar.dma_start(out[:, P:2 * P], prod_s[:, P:2 * P]).then_inc(
                    sem_out_dma, 16
                )

            @block.vector
            def _(vector):
                vector.memset(warm_s[:], 0.0).then_inc(sem_warm, 1)
                # masked products: prod = gram .* bigw, per PSUM half.
                # The first half runs while the PE still streams the
                # gram2 windows; the second right after the last fence.
                # sem_pe1 first: it can only fire mid-stream, long after
                # the sync engine's entry clears have landed, so the
                # sem_const wait that follows can never pass on a stale
                # pre-clear value.
                vector.wait_ge(sem_pe1, 1)
                vector.wait_ge(sem_const, 16)
                vector.tensor_mul(
                    prod_s[:, 0:P], gram_ps[:], bigw_s[:, 0:P]
                ).then_inc(sem_prod, 1)
                vector.wait_ge(sem_pe2, 1)
                vector.tensor_mul(
                    prod_s[:, P:2 * P], gram2_ps[:], bigw_s[:, P:2 * P]
                ).then_inc(sem_prod, 1)

            @block.tensor
            def _(tensor):
                DR = mybir.MatmulPerfMode.DoubleRow if DOUBLE_ROW else None

                def drw(ap2d):
                    # [p, 256] window -> [p, 2, 128] DoubleRow operand
                    return ap2d.rearrange("p (two f) -> p two f", two=2)

                def gram_mms(dst, src, kt, start0, stop_last, col0=0):
                    """Emit the Gram matmuls for kt rays/partition of tile
                    `src` starting at column col0 (kt % 4 == 0).  DoubleRow
                    eats 8 rays per partition per MM; a 4-ray remainder
                    gets a normal 128-col window."""
                    insts = []
                    if DOUBLE_ROW:
                        n_dr, rem = kt // 8, kt % 8
                    else:
                        n_dr, rem = 0, kt
                    n_win = n_dr + rem // 4
                    w = n_dr * 2
                    for i in range(n_dr):
                        win = src[:, col0 + i * 256:col0 + (i + 1) * 256]
                        insts.append(nc.tensor.matmul(
                            dst,
                            drw(win), drw(win),
                            start=(start0 and i == 0),
                            stop=(stop_last and i == n_win - 1),
                            perf_mode=DR,
                        ))
                    for j in range(rem // 4):
                        win = src[:, col0 + (w + j) * 128:col0 + (w + j + 1) * 128]
                        insts.append(nc.tensor.matmul(
                            dst,
                            win, win,
                            start=(start0 and n_dr == 0 and j == 0),
                            stop=(stop_last and n_dr + j == n_win - 1),
                        ))
                    return insts

                tensor.wait_ge(sem_warm, 1)
                for _ in range(WARM_MMS):
                    nc.tensor.matmul(
                        warm_ps[:], warm_s[:, 0:128], warm_s[:],
                        start=True, stop=True,
                    )
                # main stream: tiles 0..T-2 -> gram_ps
                for t in range(T - 1):
                    tensor.wait_ge(sem_tile[t], 16)
                    gram_mms(
                        gram_ps[:], bslots[t], TILE_KS[t],
                        start0=(t == 0), stop_last=(t == T - 2),
                    )
                # last tile + 32-partition tail tile -> gram2_ps.
                # A matmul's then_inc / an engine drain can fire before its
                # systolic write-back lands in PSUM (observed: torn/partial
                # reads on the DVE).  MMs complete strictly in pc order, so
                # a sem inc attached >= 2 matmuls later is a sound PSUM
                # fence: a cheap N=64 warm matmul inserted after two gram2
                # windows fences the whole main Gram for ~30ns of stream.
                tensor.wait_ge(sem_tile[T - 1], 16)
                assert TILE_KS[T - 1] >= 24
                gram_mms(
                    gram2_ps[:], bslots[T - 1], 16,
                    start0=True, stop_last=False,
                )
                nc.tensor.matmul(
                    warm_ps[:, 0:64], warm_s[:, 0:128], warm_s[:, 0:64],
                    start=True, stop=True,
                ).then_inc(sem_pe1, 1)
                gram_mms(
                    gram2_ps[:], bslots[T - 1], TILE_KS[T - 1] - 16,
                    start0=False, stop_last=False, col0=16 * BINS,
                )
                tensor.wait_ge(sem_tail, 16)
                gram_mms(
                    gram2_ps[:], tail_s, TAIL_K,
                    start0=False, stop_last=True,
                )
                # fence for gram2: two small MMs, the second carries the inc
                nc.tensor.matmul(
                    warm_ps[:, 0:64], warm_s[:, 0:128], warm_s[:, 0:64],
                    start=True, stop=True,
                )
                nc.tensor.matmul(
                    warm_ps[:, 0:64], warm_s[:, 0:128], warm_s[:, 0:64],
                    start=True, stop=True,
                ).then_inc(sem_pe2, 1)

        # No explicit receipt wait for the prod store: the scalar engine's
        # Block-exit DRAIN fences its HWDGE queue (outstanding DMAs must
        # retire before the drain completes), which saves the ~0.9us
        # completion-semaphore propagation on the critical path.  The
        # compiler-emitted NEFF teardown clears every semaphore, so no
        # explicit epilogue clears are needed either.

    return nc


def kernel(ws: np.ndarray) -> np.ndarray:
    import ml_dtypes
    from concourse.bass_utils import run_bass_kernel_spmd

    global _COMPILED, LAST_RESULTS
    if _COMPILED is None:
        _COMPILED = _build()
    nc = _COMPILED

    ws = np.asarray(ws)
    assert ws.shape == (N_RAYS, BINS), ws.shape
    # round once on the host: the device computes in this dtype anyway, and
    # streaming f32 from HBM would be excess traffic
    hdt = ml_dtypes.bfloat16 if DTYPE == "bf16" else ml_dtypes.float8_e4m3
    wsq = np.ascontiguousarray(ws).astype(hdt)
    shards = wsq.reshape(N_CORES, N_SHARD, BINS)
    in_maps = [{"ws": shards[c]} for c in range(N_CORES)]
    res = run_bass_kernel_spmd(
        nc, in_maps, list(range(N_CORES)), trace=TRACE, tmpdir=TRACE_TMPDIR,
        trace_cores=TRACE_CORES,
    )
    LAST_RESULTS = res
    total = np.float64(0.0)
    for c in range(N_CORES):
        total += np.sum(res.results[c]["out"].astype(np.float64))
    return np.array(total, dtype=np.float32)
